# revision 7
# baseline (speedup 1.0000x reference)
"""Adaptive bilateral filter (nn_AdaptiveFilter) on 8 TRN2 NeuronCores.

Math: out_c(p) = sum_k x_c(p+d_k) * wt_k(p) / sum_k wt_k(p)
with wt_k = E[src(k)] * CF_k,  CF_k = exp(-50*(sum_c |g_c(p+d_k)-g_c(p)|)^2),
E = exp(w0) precomputed on HOST (slot-major bf16), src = reflect (7,7)->(4,4).

KEY STRUCTURE — tap symmetry: for the mirror tap k' = 48-k,
  CF_{k'}(p) = CF_k(p - v_k)   (v_k = tap offset vector)
so the color field (sub/abs/channel-sum/exp chain — the dominant cost) is
computed for only 24 pairs + center (25 of 49 taps).  Fields are computed
515 cols wide (one-sided extension) so column shifts are free AP offsets;
row shifts (impossible for lockstep engines) are realized by SBUF->SBUF
DMAs into a shifted-chunk tile CFS, with the 1-3 missing bottom rows per
pair ("slivers") batch-computed from DMA-gathered halo rows.

Sharding: 8 cores = 2 batches x 4 row-bands of 128 rows, halo included in
each core's DRAM band (134 x 3*518 bf16, channels interleaved per row).
No collectives.

Engines: DVE: subtract (4 tap-rows), wt = CF*E, prod = x*wt;  ACT: Abs,
Derivative_Erf(sqrt(50)*s) = (2/sqrt(pi))exp(-50 s^2) from PSUM (constant
cancels in num/den);  PE: channel-sum + den/num accumulation (identity
stationary);  DMA: sync queue.
"""
import sys
sys.path.insert(0, "/opt/trn_rl_repo")
import math
import numpy as np

import concourse.bacc as bacc
import concourse.mybir as mybir
import concourse.tile as tile
from concourse.ap import AP
from concourse.bass_utils import run_bass_kernel_spmd

F32 = mybir.dt.float32
BF16 = mybir.dt.bfloat16
AF = mybir.ActivationFunctionType
OP = mybir.AluOpType

KH = KW = 7
HB = 128           # band rows
W = 512
WP = 518           # padded width per channel
CWIDTH = 3 * WP    # 1554
WF = 515           # uniform field width (cc_rel in [0,515))
CS = 516           # CF chunk stride
CW = 7 * WF        # u tile per-channel stride (3605)
WJ = KW * W        # 3584
SCALE = math.sqrt(50.0)

_CACHE = {}
DBG_WT = None   # tap-row i: dump wt[:, DBG_J0*W:(DBG_J0+3)*W] to out
DBG_J0 = 0


def _fidx(i, j):
    return 7 * i + j if i < 3 else 21 + j     # (3,3) center = 24


# sliver slots: pairs (i,j) with i<3 have 3-i sliver rows each, i-major
SLIV_BASE = {}
_s = 0
for _i in range(3):
    for _j in range(7):
        SLIV_BASE[(_i, _j)] = _s
        _s += 3 - _i
NSLIV = _s  # 42


def _v(t, dims, off=0):
    """AP keeping t's partition dim with custom free [stride, size] dims."""
    b = t[:] if not isinstance(t, AP) else t
    return AP(tensor=b.tensor, offset=b.offset + off,
              ap=[list(b.ap[0])] + [list(d) for d in dims])


def _emit(nc, tc, constp, gxp, workp, finp, psump, g_d, x_d, e_d, id_d,
          out_d):
    state = {"first": True}

    def mm(*args, **kwargs):
        inst = nc.tensor.matmul(*args, **kwargs)
        if state["first"]:
            state["first"] = False
        else:
            inst.ins.ldweights = False
        return inst

    ident = constp.tile([128, 128], BF16, tag="ident", name="ident")
    nc.sync.dma_start(ident[:], id_d.ap()[:, :])

    E = [constp.tile([HB, 4 * W], BF16, tag=f"E{t}", name=f"E{t}")
         for t in range(4)]
    for t in range(4):
        nc.sync.dma_start(E[t][:], e_d.ap()[:, t * 4 * W:(t + 1) * 4 * W])

    gt3 = constp.tile([HB, CWIDTH], BF16, tag="gt3", name="gt3")
    nc.sync.dma_start(gt3[:], g_d.ap()[3:3 + HB, :])

    CF = constp.tile([HB, 25 * CS], BF16, tag="CF", name="CF")
    CFS = constp.tile([HB, 21 * W], BF16, tag="CFS", name="CFS")
    gA = constp.tile([NSLIV, 3 * W], BF16, tag="gA", name="gA")
    gB = constp.tile([NSLIV, 3 * W], BF16, tag="gB", name="gB")
    sCF = constp.tile([NSLIV, W], BF16, tag="sCF", name="sCF")
    uS = constp.tile([NSLIV, 3 * W], BF16, tag="uS", name="uS")

    # center tap: s == 0, and Derivative_Erf carries a 2/sqrt(pi) factor
    # that must be uniform across taps to cancel in num/den
    nc.gpsimd.memset(CF[:, 24 * CS:25 * CS], 2.0 / math.sqrt(math.pi))

    # sliver halo gathers: pair (i,j) sliver row rr (rr < 3-i) needs
    #   gA: gp(q+v) = g_d row 128+i+rr, col offset 3   (j-independent)
    #   gB: gp(q)   = g_d row 131+rr,   col offset 6-j
    for i in range(3):
        nr = 3 - i
        base = SLIV_BASE[(i, 0)]
        cnt = 7 * nr
        bA = g_d.ap()[128 + i:128 + i + nr, :]
        nc.sync.dma_start(
            gA[base:base + cnt, :],
            AP(tensor=bA.tensor, offset=bA.offset + 3,
               ap=[[0, 7], list(bA.ap[0]), [WP, 3], [1, W]]))
        for j in range(7):
            s0 = SLIV_BASE[(i, j)]
            bB = g_d.ap()[131:131 + nr, :]
            nc.sync.dma_start(
                gB[s0:s0 + nr, :],
                AP(tensor=bB.tensor, offset=bB.offset + 6 - j,
                   ap=[list(bB.ap[0]), [WP, 3], [1, W]]))

    den_ps = psump.tile([HB, W], F32, tag="dps", name="dps", bufs=1)
    num_wide = psump.tile([HB, 3 * W], F32, tag="npsw", name="npsw",
                          bufs=1)

    udict = {}
    wtdict = {}

    def emit_sub(i):
        if i < 3:
            gt = gxp.tile([HB, CWIDTH], BF16, tag="gt", name="gt", bufs=2)
            nc.sync.dma_start(gt[:], g_d.ap()[i:i + HB, :])
        else:
            gt = gt3
        u = workp.tile([HB, 3 * CW], BF16, tag="u", name="u", bufs=2)
        # fields j in [0,3): patch g_d col offset j (stride 1/j), center off 3
        nc.vector.tensor_tensor(
            _v(u, [[CW, 3], [WF, 3], [1, WF]]),
            _v(gt, [[WP, 3], [1, 3], [1, WF]]),
            _v(gt3, [[WP, 3], [0, 3], [1, WF]], 3),
            OP.subtract)
        if i < 3:
            # fields j in [3,7): patch offset 3 const, center offset 3..0
            nc.vector.tensor_tensor(
                _v(u, [[CW, 3], [WF, 4], [1, WF]], 3 * WF),
                _v(gt, [[WP, 3], [0, 4], [1, WF]], 3),
                _v(gt3, [[WP, 3], [-1, 4], [1, WF]], 3),
                OP.subtract)
        udict[i] = u

    def emit_abs(i):
        u = udict[i]
        a = _v(u, [[CW, 3], [1, 3 * WF]])
        nc.scalar.activation(a, a, AF.Abs)
        if i < 3:
            b = _v(u, [[CW, 3], [1, 4 * WF]], 3 * WF)
            nc.scalar.activation(b, b, AF.Abs)

    def emit_csum(i):
        u = udict.pop(i)
        for j in range(7 if i < 3 else 3):
            f = _fidx(i, j)
            sA = psump.tile([HB, W], F32, tag="sA", name="sA", bufs=2)
            for c in range(3):
                mm(sA[:], ident[:],
                   u[:, c * CW + j * WF:c * CW + j * WF + W],
                   start=(c == 0), stop=(c == 2))
            sB = psump.tile([HB, 8], F32, tag="sB", name="sB", bufs=2)
            for c in range(3):
                mm(sB[:, 0:3], ident[:],
                   u[:, c * CW + j * WF + W:c * CW + (j + 1) * WF],
                   start=(c == 0), stop=(c == 2))
            nc.scalar.activation(CF[:, f * CS:f * CS + W], sA[:],
                                 AF.Derivative_Erf, scale=SCALE)
            nc.scalar.activation(CF[:, f * CS + W:f * CS + WF], sB[:, 0:3],
                                 AF.Derivative_Erf, scale=SCALE)

    def emit_shift(ic):
        """Build CFS chunks for consuming tap-row ic in {4,5,6} from field
        row i = 6-ic (+ slivers)."""
        i = 6 - ic
        npart = 131 - ic
        q0 = 7 * (ic - 4)
        b = CF[ic - 3:HB, :]
        # main parts, consuming j in [0,3): src field (i,6-j) cc_rel [0,512)
        nc.sync.dma_start(
            CFS[0:npart, q0 * W:(q0 + 3) * W],
            AP(tensor=b.tensor, offset=b.offset + (7 * i + 6) * CS,
               ap=[list(b.ap[0]), [-CS, 3], [1, W]]))
        # consuming j in [3,7): src field (i,6-j) cc_rel [j-3, 509+j)
        nc.sync.dma_start(
            CFS[0:npart, (q0 + 3) * W:(q0 + 7) * W],
            AP(tensor=b.tensor, offset=b.offset + (7 * i + 3) * CS,
               ap=[list(b.ap[0]), [-(CS - 1), 4], [1, W]]))
        # sliver rows
        nr = 3 - i
        for j in range(7):
            s0 = SLIV_BASE[(i, 6 - j)]
            nc.sync.dma_start(
                CFS[npart:HB, (q0 + j) * W:(q0 + j + 1) * W],
                sCF[s0:s0 + nr, :])

    def emit_wt(i):
        ri = min(i, 6 - i)
        eb = E[ri][:]
        wt = workp.tile([HB, WJ], BF16, tag="wt", name="wt", bufs=2)
        splits = [(0, 3), (3, 4)] if i == 3 else [(0, 4), (4, 3)]
        for (j0, nj) in splits:
            if i < 3:
                cfv = (_v(CF, [[CS, nj], [1, W]], (7 * i) * CS) if j0 == 0
                       else _v(CF, [[CS + 1, nj], [1, W]],
                               (7 * i + 4) * CS + 1))
            elif i == 3:
                cfv = (_v(CF, [[CS, nj], [1, W]], 21 * CS) if j0 == 0
                       else _v(CF, [[-(CS - 1), nj], [1, W]], 24 * CS))
            else:
                cfv = _v(CFS, [[W, nj], [1, W]], (7 * (i - 4) + j0) * W)
            if j0 == 0:
                ev = _v(eb, [[W, nj], [1, W]])
            else:
                # rj = min(j,6-j) descends from min(j0, 6-j0)
                ev = _v(eb, [[-W, nj], [1, W]], min(j0, 6 - j0) * W)
            nc.vector.tensor_tensor(
                _v(wt, [[W, nj], [1, W]], j0 * W), cfv, ev, OP.mult)
        wtdict[i] = wt
        if DBG_WT == i:
            nc.sync.dma_start(out_d.ap()[:, :],
                              wt[:, DBG_J0 * W:(DBG_J0 + 3) * W])

    def emit_cons(i):
        wt = wtdict.pop(i)
        xt = gxp.tile([HB, CWIDTH], BF16, tag="xt", name="xt", bufs=2)
        nc.sync.dma_start(xt[:], x_d.ap()[i:i + HB, :])
        first_i, last_i = (i == 0), (i == 6)
        for j in range(KW):
            mm(den_ps[:], ident[:], wt[:, j * W:(j + 1) * W],
               start=(first_i and j == 0), stop=(last_i and j == 6))
        prod = workp.tile([HB, 3 * WJ], BF16, tag="pr", name="pr", bufs=2)
        nc.vector.tensor_tensor(
            prod[:].rearrange("p (c n w) -> p c n w", c=3, n=KW),
            _v(xt, [[WP, 3], [1, KW], [1, W]]),
            _v(wt, [[0, 3], [W, KW], [1, W]]),
            OP.mult)
        for c in range(3):
            for j in range(KW):
                mm(num_wide[:, c * W:(c + 1) * W], ident[:],
                   prod[:, c * WJ + j * W:c * WJ + (j + 1) * W],
                   start=(first_i and j == 0), stop=(last_i and j == 6))

    # ---- schedule ----
    emit_sub(0)
    nc.vector.tensor_tensor(uS[:], gA[:], gB[:], OP.subtract)
    nc.scalar.activation(uS[:], uS[:], AF.Abs)
    emit_sub(1)
    emit_abs(0)
    psS = psump.tile([HB, W], F32, tag="sA", name="psS", bufs=2)
    for c in range(3):
        mm(psS[0:NSLIV, :], ident[0:NSLIV, 0:NSLIV],
           uS[:, c * W:(c + 1) * W], start=(c == 0), stop=(c == 2))
    nc.scalar.activation(sCF[:], psS[0:NSLIV, :], AF.Derivative_Erf,
                         scale=SCALE)
    emit_sub(2)
    emit_abs(1)
    emit_csum(0)
    emit_sub(3)
    emit_abs(2)
    emit_shift(6)
    emit_wt(0)
    emit_cons(0)
    emit_csum(1)
    emit_abs(3)
    emit_shift(5)
    emit_wt(1)
    emit_cons(1)
    emit_csum(2)
    emit_shift(4)
    emit_wt(2)
    emit_cons(2)
    emit_csum(3)
    for i in range(3, 7):
        emit_wt(i)
        emit_cons(i)

    rec = finp.tile([HB, W], F32, tag="rec", name="rec")
    # den in [~4e-3, ~60]: approx_fast's ~51 ULP is negligible vs bf16 noise
    nc.vector.reciprocal_approx_fast(rec[:], den_ps[:])
    o = finp.tile([HB, 3 * W], BF16, tag="o", name="o")
    nc.vector.tensor_tensor(
        o[:].rearrange("p (c w) -> p c w", c=3),
        num_wide[:].rearrange("p (c w) -> p c w", c=3),
        _v(rec, [[0, 3], [1, W]]), OP.mult)
    if DBG_WT is None:
        nc.sync.dma_start(out_d.ap()[:, :], o[:])


def _build():
    nc = bacc.Bacc("TRN2", target_bir_lowering=False, debug=False)
    g_d = nc.dram_tensor("g", [134, CWIDTH], BF16, kind="ExternalInput")
    x_d = nc.dram_tensor("x", [134, CWIDTH], BF16, kind="ExternalInput")
    e_d = nc.dram_tensor("e", [HB, 16 * W], BF16, kind="ExternalInput")
    id_d = nc.dram_tensor("ident", [128, 128], BF16, kind="ExternalInput")
    out_d = nc.dram_tensor("out", [HB, 3 * W], BF16,
                           kind="ExternalOutput")

    with tile.TileContext(nc) as tc:
        with (
            tc.tile_pool(name="const", bufs=1) as constp,
            tc.tile_pool(name="gx", bufs=2) as gxp,
            tc.tile_pool(name="work", bufs=2) as workp,
            tc.tile_pool(name="fin", bufs=1) as finp,
            tc.tile_pool(name="psum", bufs=1, space="PSUM") as psump,
        ):
            _emit(nc, tc, constp, gxp, workp, finp, psump,
                  g_d, x_d, e_d, id_d, out_d)

    nc.compile()
    return nc


def _shard_inputs(x, guidance, w0):
    import ml_dtypes
    BF = ml_dtypes.bfloat16
    pad = ((0, 0), (0, 0), (3, 3), (3, 3))
    # (B,3,518,518) -> per-core rows with channels interleaved per row
    xp = np.pad(x, pad, mode="reflect").astype(BF).transpose(0, 2, 1, 3)
    gp = np.pad(guidance, pad, mode="reflect").astype(BF).transpose(0, 2, 1, 3)
    ident = np.eye(128, dtype=BF)

    in_maps = []
    for c in range(8):
        b, band = divmod(c, 4)
        r0 = band * HB
        wslice = w0[b, r0 * W:(r0 + HB) * W]          # (65536, 4, 4)
        e = np.exp(wslice.reshape(HB, W, 4, 4).transpose(0, 2, 3, 1))
        in_maps.append({
            "g": np.ascontiguousarray(
                gp[b, r0:r0 + HB + 6].reshape(HB + 6, CWIDTH)),
            "x": np.ascontiguousarray(
                xp[b, r0:r0 + HB + 6].reshape(HB + 6, CWIDTH)),
            "e": np.ascontiguousarray(e.reshape(HB, 16 * W)).astype(BF),
            "ident": ident,
        })
    return in_maps


def kernel(x, guidance, w0):
    x = np.asarray(x, dtype=np.float32)
    guidance = np.asarray(guidance, dtype=np.float32)
    w0 = np.asarray(w0, dtype=np.float32)
    B, C, H, Wf = x.shape

    if "nc" not in _CACHE:
        _CACHE["nc"] = _build()
    nc = _CACHE["nc"]

    in_maps = _shard_inputs(x, guidance, w0)
    res = run_bass_kernel_spmd(nc, in_maps, core_ids=list(range(8)))

    out = np.empty((B, C, H, Wf), dtype=np.float32)
    for c in range(8):
        b, band = divmod(c, 4)
        r0 = band * HB
        blk = res.results[c]["out"].astype(np.float32).reshape(
            HB, 3, Wf).transpose(1, 0, 2)
        out[b, :, r0:r0 + HB, :] = blk
    return out
